# revision 12
# baseline (speedup 1.0000x reference)
"""GQA attention kernel for 8 Trainium2 NeuronCores (Bass/Tile).

Sharding: data-parallel over batch (2) x sequence-parallel over query blocks.
Core c = (b, s): batch b = c//4, slice s = c%4 owns query blocks
J = [s, 7-s, 8+s, 15-s] (128 rows each; causal work is exactly balanced:
sum of block costs = 34 for every s). Each core computes K/V for the full
sequence (cheap duplication) -> zero collectives; output is a pure
concatenation on the host.

The program is core-uniform (SPMD). Per slot v the off-diagonal key-block
loop is padded to PBAR[v]-1 = [3,7,11,15] blocks; out-of-range blocks are
killed by a per-core exp bias table (exp(s/8 - 50) ~ 0). The causal
diagonal block is a separate fixed iteration whose K/V come from an extra
projection of the per-core gathered query columns (xqT), so its mask is a
core-independent affine_select.

All weights (transposed, bf16), x (transposed, bf16), gathered q-columns,
RoPE cos/sin tables (expanded, signed) and the pair-swap matrix are
prepared host-side in run().

Hardcoded problem: B=2 T=2048 D=1024 n_heads=16 n_kv=4 d_head=64, causal,
rope passed as input, scale=1/8.
"""

import numpy as np
import ml_dtypes

import concourse.bass as bass
import concourse.tile as tile
from concourse import bacc, mybir
from concourse.bass_utils import run_bass_kernel_spmd

F32 = mybir.dt.float32
BF16 = mybir.dt.bfloat16

B, T, D = 2, 2048, 1024
NH, NKV, DH = 16, 4, 64
NTB = T // 128            # 16 key blocks
PBAR = [4, 8, 12, 16]     # total key-block count per slot (uniform program)
SCALE = 1.0 / 8.0
NEG = -50.0               # exp bias for out-of-range blocks

# head permutation: q-chunk c holds heads (PI[2c], PI[2c+1]) in its two
# 64-partition halves -> each chunk pairs a g-even head with a g-odd head so
# scores can be row-tiled (two concurrent K=64 matmuls) against the matching
# kv-head pair.
PI = [0, 4, 1, 5, 2, 6, 3, 7, 8, 12, 9, 13, 10, 14, 11, 15]

USE_APPROX_RECIP = True

_CACHE = {}


def _emit(nc, tc, aps):
    (xT_ap, xqT_ap, wqT_ap, wkT_ap, wvT_ap, woT_ap,
     c2k_ap, s2k_ap, c2q_ap, s2q_ap, swap_ap, bias_ap, out_ap) = aps
    import contextlib
    ctx = contextlib.ExitStack()
    with ctx:
        sing = ctx.enter_context(tc.tile_pool(name="sing", bufs=1))
        qsbp = ctx.enter_context(tc.tile_pool(name="qsbp", bufs=4))
        ropet = ctx.enter_context(tc.tile_pool(name="ropet", bufs=4))
        ptp = ctx.enter_context(tc.tile_pool(name="ptp", bufs=4))
        rp = ctx.enter_context(tc.tile_pool(name="rp", bufs=4))
        osp = ctx.enter_context(tc.tile_pool(name="osp", bufs=3))
        # PSUM: tp(3x2 banks) + oap(2x1) = 8 banks
        tp = ctx.enter_context(tc.tile_pool(name="tp", bufs=3, space="PSUM"))
        oap = ctx.enter_context(tc.tile_pool(name="oap", bufs=2, space="PSUM"))

        # ---- persistent SBUF tensors; DMAs emitted in dependency priority
        xT = sing.tile([128, 8, T], BF16)
        wkT = sing.tile([128, 8, 256], BF16)
        wvT = sing.tile([128, 8, 256], BF16)
        wqT = sing.tile([128, 8, 1024], BF16)
        woT = sing.tile([64, 16, 1024], BF16)
        xqT = sing.tile([128, 8, 512], BF16)
        c2k = sing.tile([128, T], BF16)
        s2k = sing.tile([128, T], BF16)
        c2q = sing.tile([128, 512], BF16)
        s2q = sing.tile([128, 512], BF16)
        swap = sing.tile([128, 128], BF16)
        bias = sing.tile([128, 4, 16], F32)

        nc.sync.dma_start(wkT[:], wkT_ap.rearrange("(m p) o -> p m o", p=128))
        nc.sync.dma_start(swap[:], swap_ap[:, :])
        nc.sync.dma_start(c2k[:, 0:512], c2k_ap[:, 0:512])
        nc.sync.dma_start(s2k[:, 0:512], s2k_ap[:, 0:512])
        for m in range(8):  # ts0 columns of x first
            nc.sync.dma_start(xT[:, m, 0:512], xT_ap[128 * m:128 * (m + 1), 0:512])
        nc.sync.dma_start(xqT[:], xqT_ap.rearrange("(m p) t -> p m t", p=128))
        nc.sync.dma_start(c2q[:], c2q_ap[:, :])
        nc.sync.dma_start(s2q[:], s2q_ap[:, :])
        nc.sync.dma_start(wvT[:], wvT_ap.rearrange("(m p) o -> p m o", p=128))
        nc.sync.dma_start(wqT[:], wqT_ap.rearrange("(m p) o -> p m o", p=128))
        nc.sync.dma_start(bias[:], bias_ap.rearrange("p (v i) -> p v i", v=4))
        for ts in range(1, 4):
            for m in range(8):
                nc.sync.dma_start(xT[:, m, 512 * ts:512 * (ts + 1)],
                                  xT_ap[128 * m:128 * (m + 1),
                                        512 * ts:512 * (ts + 1)])
            nc.sync.dma_start(c2k[:, 512 * ts:512 * (ts + 1)],
                              c2k_ap[:, 512 * ts:512 * (ts + 1)])
            nc.sync.dma_start(s2k[:, 512 * ts:512 * (ts + 1)],
                              s2k_ap[:, 512 * ts:512 * (ts + 1)])
        nc.sync.dma_start(woT[:], woT_ap.rearrange("(h p) n -> p h n", p=64))

        kroped = sing.tile([128, 2, T], BF16)     # [kv-pair half, P, t]
        kdiag = sing.tile([128, 2, 4, 128], BF16)  # [half, P, slot, tq]
        qTd = sing.tile([128, 8, 4, 128], BF16)   # [head-pair half, chunk, slot, tq]
        vaug = sing.tile([128, NTB, 4, 65], BF16)  # [tk, tb, g, dh | ones]
        nc.vector.memset(vaug[:], 1.0)
        vdiag = sing.tile([128, 4, 4, 65], BF16)  # [tk, slot, g, dh | ones]
        nc.vector.memset(vdiag[:], 1.0)
        oaTn = sing.tile([64, 16, 4, 128], BF16)  # [dh, head, slot, tq]

        def rope(kt, dst, cos_ap, sin_ap):
            """kt: [128, 1024] psum tile; raw proj in [:, 0:512], the
            swap-matmul result goes into [:, 512:1024].
            dst/cos/sin: [128, 512] bf16 views."""
            qsb = qsbp.tile([128, 512], BF16, tag="qsb")
            nc.scalar.copy(qsb[:], kt[:, 0:512])
            nc.tensor.matmul(kt[:, 512:1024], swap[:], qsb[:],
                             start=True, stop=True)
            t1 = ropet.tile([128, 512], BF16, tag="t1")
            nc.gpsimd.tensor_mul(t1[:], qsb[:], cos_ap)
            t2 = ropet.tile([128, 512], F32, tag="t2")
            nc.vector.tensor_mul(t2[:], kt[:, 512:1024], sin_ap)
            nc.vector.tensor_add(dst, t1[:], t2[:])

        def kproj(och, ts):
            kt = tp.tile([128, 1024], F32, tag="t")
            for m in range(8):
                nc.tensor.matmul(kt[:, 0:512], wkT[:, m, 128 * och:128 * (och + 1)],
                                 xT[:, m, 512 * ts:512 * (ts + 1)],
                                 start=(m == 0), stop=(m == 7))
            rope(kt, kroped[:, och, 512 * ts:512 * (ts + 1)],
                 c2k[:, 512 * ts:512 * (ts + 1)],
                 s2k[:, 512 * ts:512 * (ts + 1)])

        def vproj(tb):
            vt = tp.tile([128, 1024], F32, tag="t")
            for m in range(8):
                nc.tensor.matmul(vt[:, 0:256], xT[:, m, 128 * tb:128 * (tb + 1)],
                                 wvT[:, m, :], start=(m == 0), stop=(m == 7))
            nc.scalar.copy(vaug[:, tb, :, 0:64],
                           vt[:, 0:256].rearrange("p (g d) -> p g d", g=4))

        # ---- K(ts0) + diagonal K/V + Q projections
        kproj(0, 0)
        kproj(1, 0)
        for och in range(2):  # kdiag from gathered q-columns
            kt = tp.tile([128, 1024], F32, tag="t")
            for m in range(8):
                nc.tensor.matmul(kt[:, 0:512], wkT[:, m, 128 * och:128 * (och + 1)],
                                 xqT[:, m, :], start=(m == 0), stop=(m == 7))
            rope(kt, kdiag[:, och, :, :].rearrange("p v t -> p (v t)"),
                 c2q[:, :], s2q[:, :])
        for v in range(4):  # vdiag
            vt = tp.tile([128, 1024], F32, tag="t")
            for m in range(8):
                nc.tensor.matmul(vt[:, 0:256], xqT[:, m, 128 * v:128 * (v + 1)],
                                 wvT[:, m, :], start=(m == 0), stop=(m == 7))
            nc.scalar.copy(vdiag[:, v, :, 0:64],
                           vt[:, 0:256].rearrange("p (g d) -> p g d", g=4))
        for och in range(8):  # Q (all 4 slots at once per chunk)
            qt = tp.tile([128, 1024], F32, tag="t")
            for m in range(8):
                nc.tensor.matmul(qt[:, 0:512], wqT[:, m, 128 * och:128 * (och + 1)],
                                 xqT[:, m, :], start=(m == 0), stop=(m == 7))
            rope(qt, qTd[:, och, :, :].rearrange("p v t -> p (v t)"),
                 c2q[:, :], s2q[:, :])

        def attend(v, P):
            oaA = oap.tile([128, 512], F32, tag="oa", name=f"oaA{v}{P}")
            oaB = oap.tile([128, 512], F32, tag="oa", name=f"oaB{v}{P}")
            rhsA = qTd[0:64, 4 * P:4 * P + 4, v, :]
            rhsB = qTd[64:128, 4 * P:4 * P + 4, v, :]
            # diagonal block (own columns, fixed causal mask)
            st = tp.tile([128, 1024], F32, tag="t")
            nc.tensor.matmul(st[:, 0:512], kdiag[0:64, P, v, :], rhsA,
                             start=True, stop=True)
            nc.tensor.matmul(st[:, 512:1024], kdiag[64:128, P, v, :], rhsB,
                             start=True, stop=True, tile_position=(64, 0))
            pt = ptp.tile([128, 1024], BF16, tag="pt")
            nc.scalar.activation(pt[:], st[:], mybir.ActivationFunctionType.Exp,
                                 scale=SCALE)
            nc.gpsimd.affine_select(
                out=pt[:].rearrange("p (h c) -> p h c", h=8),
                in_=pt[:].rearrange("p (h c) -> p h c", h=8),
                compare_op=mybir.AluOpType.is_ge,
                fill=0.0, base=0, pattern=[[0, 8], [1, 128]],
                channel_multiplier=-1)
            nc.tensor.matmul(oaA[0:65, :], vdiag[:, v, 2 * P, :], pt[:, 0:512],
                             start=True, stop=False)
            nc.tensor.matmul(oaB[0:65, :], vdiag[:, v, 2 * P + 1, :],
                             pt[:, 512:1024], start=True, stop=False)
            # off-diagonal blocks (bias table kills i >= J[v])
            nb = PBAR[v] - 1
            for i in range(nb):
                st = tp.tile([128, 1024], F32, tag="t")
                nc.tensor.matmul(st[:, 0:512],
                                 kroped[0:64, P, 128 * i:128 * (i + 1)], rhsA,
                                 start=True, stop=True)
                nc.tensor.matmul(st[:, 512:1024],
                                 kroped[64:128, P, 128 * i:128 * (i + 1)], rhsB,
                                 start=True, stop=True, tile_position=(64, 0))
                pt = ptp.tile([128, 1024], BF16, tag="pt")
                nc.scalar.activation(pt[:], st[:],
                                     mybir.ActivationFunctionType.Exp,
                                     scale=SCALE, bias=bias[:, v, i:i + 1])
                nc.tensor.matmul(oaA[0:65, :], vaug[:, i, 2 * P, :],
                                 pt[:, 0:512], start=False, stop=(i == nb - 1))
                nc.tensor.matmul(oaB[0:65, :], vaug[:, i, 2 * P + 1, :],
                                 pt[:, 512:1024], start=False, stop=(i == nb - 1))
            for half, oa in ((0, oaA), (1, oaB)):
                rsb = rp.tile([1, 512], F32, tag="rsb")
                if USE_APPROX_RECIP:
                    dcp = rp.tile([1, 512], F32, tag="dcp")
                    nc.scalar.copy(dcp[:], oa[64:65, :])
                    nc.vector.reciprocal_approx_fast(out=rsb[:], in_=dcp[:])
                else:
                    nc.vector.reciprocal(rsb[:], oa[64:65, :])
                rb = rp.tile([64, 512], F32, tag="rb")
                nc.gpsimd.partition_broadcast(rb[:], rsb[:])
                h0 = 8 * P + 4 * half
                nc.vector.tensor_mul(
                    oaTn[0:64, h0:h0 + 4, v, :],
                    oa[0:64, :].rearrange("p (h c) -> p h c", h=4),
                    rb[:].rearrange("p (h c) -> p h c", h=4))

        def outproj(v):
            op = tp.tile([128, 1024], F32, tag="t")
            for nh in range(2):
                for h in range(16):
                    nc.tensor.matmul(op[:, 512 * nh:512 * (nh + 1)],
                                     oaTn[0:64, h, v, :],
                                     woT[0:64, h, 512 * nh:512 * (nh + 1)],
                                     start=(h == 0), stop=(h == 15))
            for nh in range(2):
                ob = osp.tile([128, 512], F32, tag="ob")
                nc.vector.tensor_copy(ob[:], op[:, 512 * nh:512 * (nh + 1)])
                nc.sync.dma_start(
                    out_ap[128 * v:128 * (v + 1), 512 * nh:512 * (nh + 1)], ob[:])

        # slot 0 needs K blocks 0..2 (ts0, above) and V tb 0..2
        for tb in range(3):
            vproj(tb)
        for v in range(4):
            attend(v, 0)
            attend(v, 1)
            if v < 3:  # next slice's K/V
                kproj(0, v + 1)
                kproj(1, v + 1)
                for tb in range(4 * v + 3, 4 * v + 7):
                    vproj(tb)
            outproj(v)


def _build():
    if "nc" in _CACHE:
        return _CACHE["nc"]
    nc = bacc.Bacc("TRN2", target_bir_lowering=False, debug=False, num_devices=8)
    aps = (
        nc.dram_tensor("xT", [D, T], BF16, kind="ExternalInput").ap(),
        nc.dram_tensor("xqT", [D, 512], BF16, kind="ExternalInput").ap(),
        nc.dram_tensor("wqT", [D, 1024], BF16, kind="ExternalInput").ap(),
        nc.dram_tensor("wkT", [D, 256], BF16, kind="ExternalInput").ap(),
        nc.dram_tensor("wvT", [D, 256], BF16, kind="ExternalInput").ap(),
        nc.dram_tensor("woT", [1024, 1024], BF16, kind="ExternalInput").ap(),
        nc.dram_tensor("c2k", [128, T], BF16, kind="ExternalInput").ap(),
        nc.dram_tensor("s2k", [128, T], BF16, kind="ExternalInput").ap(),
        nc.dram_tensor("c2q", [128, 512], BF16, kind="ExternalInput").ap(),
        nc.dram_tensor("s2q", [128, 512], BF16, kind="ExternalInput").ap(),
        nc.dram_tensor("swap", [128, 128], BF16, kind="ExternalInput").ap(),
        nc.dram_tensor("bias", [128, 64], F32, kind="ExternalInput").ap(),
        nc.dram_tensor("out", [512, D], F32, kind="ExternalOutput").ap(),
    )
    with tile.TileContext(nc) as tc:
        _emit(nc, tc, aps)
    nc.compile()
    _CACHE["nc"] = nc
    return nc


def _host_prep(inputs):
    """Build per-core input maps (all numpy, untimed host prep)."""
    bf = ml_dtypes.bfloat16
    x = np.asarray(inputs["x"], np.float32)
    w_q = np.asarray(inputs["w_q"], np.float32)
    w_k = np.asarray(inputs["w_k"], np.float32)
    w_v = np.asarray(inputs["w_v"], np.float32)
    w_o = np.asarray(inputs["w_o"], np.float32)
    rf = np.asarray(inputs["rope_freqs"], np.float32)  # [T, 32, 2]

    # q head permutation: chunk c = (PI[2c], PI[2c+1])
    perm_rows = np.empty(1024, np.int64)
    for c in range(8):
        for j in range(128):
            h = PI[2 * c + (1 if j >= 64 else 0)]
            perm_rows[128 * c + j] = 64 * h + (j % 64)
    wq_perm = w_q[perm_rows]

    wqT = np.ascontiguousarray(wq_perm.T).astype(bf)          # [1024d, 1024o]
    wkT = np.ascontiguousarray(w_k.T).astype(bf)              # [1024, 256]
    wvT = np.ascontiguousarray(w_v.T).astype(bf)              # [1024, 256]
    woT = np.ascontiguousarray(w_o.T).astype(bf)              # [1024o, 1024n]

    # rope tables [128, T]: partition p -> dim (p % 64) of any head
    p = np.arange(128)
    f = (p % 64) // 2
    sign = np.where(p % 2 == 1, 1.0, -1.0).astype(np.float32)
    c2k = np.ascontiguousarray(rf[:, f, 0].T).astype(bf)          # [128, T]
    s2k = np.ascontiguousarray((rf[:, f, 1] * sign).T).astype(bf)

    swap_m = np.zeros((128, 128), np.float32)
    q = (p // 64) * 64 + ((p % 64) ^ 1)
    swap_m[q, p] = 1.0
    swap_m = swap_m.astype(bf)

    in_maps = []
    for c in range(8):
        b, s = divmod(c, 4)
        J = [s, 7 - s, 8 + s, 15 - s]
        tcols = np.concatenate([np.arange(128 * j, 128 * (j + 1)) for j in J])
        xTb = np.ascontiguousarray(x[b].T).astype(bf)         # [1024, 2048]
        xqT = np.ascontiguousarray(xTb[:, tcols])             # [1024, 512]
        c2q = np.ascontiguousarray(c2k[:, tcols])
        s2q = np.ascontiguousarray(s2k[:, tcols])
        bias_t = np.zeros((4, 16), np.float32)
        for v in range(4):
            bias_t[v, J[v]:] = NEG   # off-diag loop: block i valid iff i < J[v]
        bias_full = np.ascontiguousarray(
            np.broadcast_to(bias_t.reshape(1, 64), (128, 64)))
        in_maps.append({
            "xT": xTb, "xqT": xqT, "wqT": wqT, "wkT": wkT, "wvT": wvT,
            "woT": woT, "c2k": c2k, "s2k": s2k, "c2q": c2q, "s2q": s2q,
            "swap": swap_m, "bias": bias_full,
        })
    return in_maps


def run(trace=False, tmpdir=None, **inputs):
    nc = _build()
    in_maps = _host_prep(inputs)
    res = run_bass_kernel_spmd(nc, in_maps, core_ids=list(range(8)), trace=trace,
                               tmpdir=tmpdir)
    out = np.empty((B, T, D), np.float32)
    for c in range(8):
        b, s = divmod(c, 4)
        J = [s, 7 - s, 8 + s, 15 - s]
        for v in range(4):
            out[b, 128 * J[v]:128 * (J[v] + 1)] = \
                res.results[c]["out"][128 * v:128 * (v + 1)]
    return out, res


def kernel(**inputs):
    out, _ = run(trace=False, **inputs)
    return out


# revision 14
# speedup vs baseline: 1.2761x; 1.2761x over previous
"""GQA attention kernel for 8 Trainium2 NeuronCores (Bass/Tile).

Sharding: data-parallel over batch (2) x sequence-parallel over query blocks.
Core c = (b, s): batch b = c//4, slice s = c%4 owns query blocks
J = [s, 7-s, 8+s, 15-s] (128 rows each; causal work is exactly balanced:
sum of block costs = 34 for every s). Each core computes K/V for the full
sequence (cheap duplication) -> zero collectives; output is a pure
concatenation on the host.

The program is core-uniform (SPMD). Per slot v the off-diagonal key-block
loop is padded to PBAR[v]-1 = [3,7,11,15] blocks; out-of-range blocks are
killed by a per-core exp bias table (exp(s/8 - 50) ~ 0). The causal
diagonal block is a separate fixed iteration whose K/V come from an extra
projection of the per-core gathered query columns (xqT), so its mask is a
core-independent affine_select.

All weights (transposed, bf16), x (transposed, bf16), gathered q-columns,
RoPE cos/sin tables (expanded, signed) and the pair-swap matrix are
prepared host-side in run().

Hardcoded problem: B=2 T=2048 D=1024 n_heads=16 n_kv=4 d_head=64, causal,
rope passed as input, scale=1/8.
"""

import numpy as np
import ml_dtypes

import concourse.bass as bass
import concourse.tile as tile
from concourse import bacc, mybir
from concourse.bass_utils import run_bass_kernel_spmd

F32 = mybir.dt.float32
BF16 = mybir.dt.bfloat16

B, T, D = 2, 2048, 1024
NH, NKV, DH = 16, 4, 64
NTB = T // 128            # 16 key blocks
PBAR = [4, 8, 12, 16]     # total key-block count per slot (uniform program)
SCALE = 1.0 / 8.0
NEG = -50.0               # exp bias for out-of-range blocks

# head permutation: q-chunk c holds heads (PI[2c], PI[2c+1]) in its two
# 64-partition halves -> each chunk pairs a g-even head with a g-odd head so
# scores can be row-tiled (two concurrent K=64 matmuls) against the matching
# kv-head pair.
PI = [0, 4, 1, 5, 2, 6, 3, 7, 8, 12, 9, 13, 10, 14, 11, 15]

USE_APPROX_RECIP = True

_CACHE = {}


def _emit(nc, tc, aps):
    (xT_ap, xqT_ap, wqT_ap, wkT_ap, wvT_ap, woT_ap,
     c2k_ap, s2k_ap, c2q_ap, s2q_ap, swap_ap, bias_ap, out_ap) = aps
    import contextlib
    ctx = contextlib.ExitStack()
    with ctx:
        sing = ctx.enter_context(tc.tile_pool(name="sing", bufs=1))
        qsbp = ctx.enter_context(tc.tile_pool(name="qsbp", bufs=4))
        ropet = ctx.enter_context(tc.tile_pool(name="ropet", bufs=4))
        ptp = ctx.enter_context(tc.tile_pool(name="ptp", bufs=4))
        rp = ctx.enter_context(tc.tile_pool(name="rp", bufs=2))
        osp = ctx.enter_context(tc.tile_pool(name="osp", bufs=3))
        # PSUM: tp(3x2 banks) + oap(2x1) = 8 banks
        tp = ctx.enter_context(tc.tile_pool(name="tp", bufs=3, space="PSUM"))
        oap = ctx.enter_context(tc.tile_pool(name="oap", bufs=2, space="PSUM"))

        # ---- persistent SBUF tensors; DMAs emitted in dependency priority
        xT = sing.tile([128, 8, T], BF16)
        wkT = sing.tile([128, 8, 256], BF16)
        wvT = sing.tile([128, 8, 256], BF16)
        wqT = sing.tile([128, 8, 1024], BF16)
        woT = sing.tile([64, 16, 1024], BF16)
        xqT = sing.tile([128, 8, 512], BF16)
        c2k = sing.tile([128, T], BF16)
        s2k = sing.tile([128, T], BF16)
        c2q = sing.tile([128, 512], BF16)
        s2q = sing.tile([128, 512], BF16)
        swap = sing.tile([128, 128], BF16)
        bias = sing.tile([128, 4, 16], F32)

        nc.sync.dma_start(wkT[:], wkT_ap.rearrange("(m p) o -> p m o", p=128))
        nc.sync.dma_start(swap[:], swap_ap[:, :])
        nc.sync.dma_start(c2k[:, 0:512], c2k_ap[:, 0:512])
        nc.sync.dma_start(s2k[:, 0:512], s2k_ap[:, 0:512])
        for m in range(8):  # ts0 columns of x first
            nc.sync.dma_start(xT[:, m, 0:512], xT_ap[128 * m:128 * (m + 1), 0:512])
        nc.sync.dma_start(xqT[:], xqT_ap.rearrange("(m p) t -> p m t", p=128))
        nc.sync.dma_start(c2q[:], c2q_ap[:, :])
        nc.sync.dma_start(s2q[:], s2q_ap[:, :])
        nc.sync.dma_start(wvT[:], wvT_ap.rearrange("(m p) o -> p m o", p=128))
        nc.sync.dma_start(wqT[:], wqT_ap.rearrange("(m p) o -> p m o", p=128))
        nc.sync.dma_start(bias[:], bias_ap.rearrange("p (v i) -> p v i", v=4))
        for ts in range(1, 4):
            for m in range(8):
                nc.sync.dma_start(xT[:, m, 512 * ts:512 * (ts + 1)],
                                  xT_ap[128 * m:128 * (m + 1),
                                        512 * ts:512 * (ts + 1)])
            nc.sync.dma_start(c2k[:, 512 * ts:512 * (ts + 1)],
                              c2k_ap[:, 512 * ts:512 * (ts + 1)])
            nc.sync.dma_start(s2k[:, 512 * ts:512 * (ts + 1)],
                              s2k_ap[:, 512 * ts:512 * (ts + 1)])
        nc.sync.dma_start(woT[:], woT_ap.rearrange("(h p) n -> p h n", p=64))

        kroped = sing.tile([128, 2, T], BF16)     # [kv-pair half, P, t]
        kdiag = sing.tile([128, 2, 4, 128], BF16)  # [half, P, slot, tq]
        qTd = sing.tile([128, 8, 4, 128], BF16)   # [head-pair half, chunk, slot, tq]
        vaug = sing.tile([128, NTB, 4, 65], BF16)  # [tk, tb, g, dh | ones]
        nc.vector.memset(vaug[:], 1.0)
        vdiag = sing.tile([128, 4, 4, 65], BF16)  # [tk, slot, g, dh | ones]
        nc.vector.memset(vdiag[:], 1.0)
        oaTn = sing.tile([64, 16, 4, 128], BF16)  # [dh, head, slot, tq]
        ones64 = sing.tile([1, 64], BF16)
        nc.vector.memset(ones64[:], 1.0)

        def rope(kt, dst, cos_ap, sin_ap):
            """kt: [128, 1024] psum tile; raw proj in [:, 0:512], the
            swap-matmul result goes into [:, 512:1024].
            dst/cos/sin: [128, 512] bf16 views."""
            qsb = qsbp.tile([128, 512], BF16, tag="qsb")
            nc.scalar.copy(qsb[:], kt[:, 0:512])
            nc.tensor.matmul(kt[:, 512:1024], swap[:], qsb[:],
                             start=True, stop=True)
            t1 = ropet.tile([128, 512], BF16, tag="t1")
            nc.vector.tensor_mul(t1[:], qsb[:], cos_ap)
            t2 = ropet.tile([128, 512], F32, tag="t2")
            nc.vector.tensor_mul(t2[:], kt[:, 512:1024], sin_ap)
            nc.vector.tensor_add(dst, t1[:], t2[:])

        def kproj(och, ts):
            kt = tp.tile([128, 1024], F32, tag="t")
            for m in range(8):
                nc.tensor.matmul(kt[:, 0:512], wkT[:, m, 128 * och:128 * (och + 1)],
                                 xT[:, m, 512 * ts:512 * (ts + 1)],
                                 start=(m == 0), stop=(m == 7))
            rope(kt, kroped[:, och, 512 * ts:512 * (ts + 1)],
                 c2k[:, 512 * ts:512 * (ts + 1)],
                 s2k[:, 512 * ts:512 * (ts + 1)])

        def vproj(tb):
            vt = tp.tile([128, 1024], F32, tag="t")
            for m in range(8):
                nc.tensor.matmul(vt[:, 0:256], xT[:, m, 128 * tb:128 * (tb + 1)],
                                 wvT[:, m, :], start=(m == 0), stop=(m == 7))
            nc.scalar.copy(vaug[:, tb, :, 0:64],
                           vt[:, 0:256].rearrange("p (g d) -> p g d", g=4))

        # ---- K(ts0) + diagonal K/V + Q projections
        kproj(0, 0)
        kproj(1, 0)
        for och in range(2):  # kdiag from gathered q-columns
            kt = tp.tile([128, 1024], F32, tag="t")
            for m in range(8):
                nc.tensor.matmul(kt[:, 0:512], wkT[:, m, 128 * och:128 * (och + 1)],
                                 xqT[:, m, :], start=(m == 0), stop=(m == 7))
            rope(kt, kdiag[:, och, :, :].rearrange("p v t -> p (v t)"),
                 c2q[:, :], s2q[:, :])
        for v in range(4):  # vdiag
            vt = tp.tile([128, 1024], F32, tag="t")
            for m in range(8):
                nc.tensor.matmul(vt[:, 0:256], xqT[:, m, 128 * v:128 * (v + 1)],
                                 wvT[:, m, :], start=(m == 0), stop=(m == 7))
            nc.scalar.copy(vdiag[:, v, :, 0:64],
                           vt[:, 0:256].rearrange("p (g d) -> p g d", g=4))
        for och in range(8):  # Q (all 4 slots at once per chunk)
            qt = tp.tile([128, 1024], F32, tag="t")
            for m in range(8):
                nc.tensor.matmul(qt[:, 0:512], wqT[:, m, 128 * och:128 * (och + 1)],
                                 xqT[:, m, :], start=(m == 0), stop=(m == 7))
            rope(qt, qTd[:, och, :, :].rearrange("p v t -> p (v t)"),
                 c2q[:, :], s2q[:, :])

        def attend(v, P):
            oaA = oap.tile([128, 512], F32, tag="oa", name=f"oaA{v}{P}")
            oaB = oap.tile([128, 512], F32, tag="oa", name=f"oaB{v}{P}")
            rhsA = qTd[0:64, 4 * P:4 * P + 4, v, :]
            rhsB = qTd[64:128, 4 * P:4 * P + 4, v, :]
            # diagonal block (own columns, fixed causal mask)
            st = tp.tile([128, 1024], F32, tag="t")
            nc.tensor.matmul(st[:, 0:512], kdiag[0:64, P, v, :], rhsA,
                             start=True, stop=True)
            nc.tensor.matmul(st[:, 512:1024], kdiag[64:128, P, v, :], rhsB,
                             start=True, stop=True, tile_position=(64, 0))
            pt = ptp.tile([128, 1024], BF16, tag="pt")
            nc.scalar.activation(pt[:], st[:], mybir.ActivationFunctionType.Exp,
                                 scale=SCALE)
            nc.gpsimd.affine_select(
                out=pt[:].rearrange("p (h c) -> p h c", h=8),
                in_=pt[:].rearrange("p (h c) -> p h c", h=8),
                compare_op=mybir.AluOpType.is_ge,
                fill=0.0, base=0, pattern=[[0, 8], [1, 128]],
                channel_multiplier=-1)
            nc.tensor.matmul(oaA[0:65, :], vdiag[:, v, 2 * P, :], pt[:, 0:512],
                             start=True, stop=False)
            nc.tensor.matmul(oaB[0:65, :], vdiag[:, v, 2 * P + 1, :],
                             pt[:, 512:1024], start=True, stop=False)
            # off-diagonal blocks (bias table kills i >= J[v])
            nb = PBAR[v] - 1
            for i in range(nb):
                st = tp.tile([128, 1024], F32, tag="t")
                nc.tensor.matmul(st[:, 0:512],
                                 kroped[0:64, P, 128 * i:128 * (i + 1)], rhsA,
                                 start=True, stop=True)
                nc.tensor.matmul(st[:, 512:1024],
                                 kroped[64:128, P, 128 * i:128 * (i + 1)], rhsB,
                                 start=True, stop=True, tile_position=(64, 0))
                pt = ptp.tile([128, 1024], BF16, tag="pt")
                nc.scalar.activation(pt[:], st[:],
                                     mybir.ActivationFunctionType.Exp,
                                     scale=SCALE, bias=bias[:, v, i:i + 1])
                nc.tensor.matmul(oaA[0:65, :], vaug[:, i, 2 * P, :],
                                 pt[:, 0:512], start=False, stop=(i == nb - 1))
                nc.tensor.matmul(oaB[0:65, :], vaug[:, i, 2 * P + 1, :],
                                 pt[:, 512:1024], start=False, stop=(i == nb - 1))
            for half, oa in ((0, oaA), (1, oaB)):
                rsbb = rp.tile([1, 512], BF16, tag="rsbb")
                if USE_APPROX_RECIP:
                    dcp = rp.tile([1, 512], F32, tag="dcp")
                    nc.scalar.copy(dcp[:], oa[64:65, :])
                    rsb = rp.tile([1, 512], F32, tag="rsb")
                    nc.vector.reciprocal_approx_fast(out=rsb[:], in_=dcp[:])
                    nc.vector.tensor_copy(rsbb[:], rsb[:])
                else:
                    nc.vector.reciprocal(rsbb[:], oa[64:65, :])
                rbp = tp.tile([128, 1024], F32, tag="t", name=f"rb{v}{P}{half}")
                nc.tensor.matmul(rbp[0:64, 0:512], ones64[:], rsbb[:],
                                 start=True, stop=True)
                rb = rp.tile([64, 512], F32, tag="rb")
                nc.vector.tensor_copy(rb[:], rbp[0:64, 0:512])
                h0 = 8 * P + 4 * half
                nc.vector.tensor_mul(
                    oaTn[0:64, h0:h0 + 4, v, :],
                    oa[0:64, :].rearrange("p (h c) -> p h c", h=4),
                    rb[:].rearrange("p (h c) -> p h c", h=4))

        def outproj(v):
            op = tp.tile([128, 1024], F32, tag="t")
            for nh in range(2):
                for h in range(16):
                    nc.tensor.matmul(op[:, 512 * nh:512 * (nh + 1)],
                                     oaTn[0:64, h, v, :],
                                     woT[0:64, h, 512 * nh:512 * (nh + 1)],
                                     start=(h == 0), stop=(h == 15))
            for nh in range(2):
                ob = osp.tile([128, 512], F32, tag="ob")
                nc.vector.tensor_copy(ob[:], op[:, 512 * nh:512 * (nh + 1)])
                nc.sync.dma_start(
                    out_ap[128 * v:128 * (v + 1), 512 * nh:512 * (nh + 1)], ob[:])

        # slot 0 needs K blocks 0..2 (ts0, above) and V tb 0..2
        for tb in range(3):
            vproj(tb)
        for v in range(4):
            attend(v, 0)
            attend(v, 1)
            if v < 3:  # next slice's K/V
                kproj(0, v + 1)
                kproj(1, v + 1)
                for tb in range(4 * v + 3, 4 * v + 7):
                    vproj(tb)
            outproj(v)


def _build():
    if "nc" in _CACHE:
        return _CACHE["nc"]
    nc = bacc.Bacc("TRN2", target_bir_lowering=False, debug=False, num_devices=8)
    aps = (
        nc.dram_tensor("xT", [D, T], BF16, kind="ExternalInput").ap(),
        nc.dram_tensor("xqT", [D, 512], BF16, kind="ExternalInput").ap(),
        nc.dram_tensor("wqT", [D, 1024], BF16, kind="ExternalInput").ap(),
        nc.dram_tensor("wkT", [D, 256], BF16, kind="ExternalInput").ap(),
        nc.dram_tensor("wvT", [D, 256], BF16, kind="ExternalInput").ap(),
        nc.dram_tensor("woT", [1024, 1024], BF16, kind="ExternalInput").ap(),
        nc.dram_tensor("c2k", [128, T], BF16, kind="ExternalInput").ap(),
        nc.dram_tensor("s2k", [128, T], BF16, kind="ExternalInput").ap(),
        nc.dram_tensor("c2q", [128, 512], BF16, kind="ExternalInput").ap(),
        nc.dram_tensor("s2q", [128, 512], BF16, kind="ExternalInput").ap(),
        nc.dram_tensor("swap", [128, 128], BF16, kind="ExternalInput").ap(),
        nc.dram_tensor("bias", [128, 64], F32, kind="ExternalInput").ap(),
        nc.dram_tensor("out", [512, D], F32, kind="ExternalOutput").ap(),
    )
    with tile.TileContext(nc) as tc:
        _emit(nc, tc, aps)
    nc.compile()
    _CACHE["nc"] = nc
    return nc


def _host_prep(inputs):
    """Build per-core input maps (all numpy, untimed host prep)."""
    bf = ml_dtypes.bfloat16
    x = np.asarray(inputs["x"], np.float32)
    w_q = np.asarray(inputs["w_q"], np.float32)
    w_k = np.asarray(inputs["w_k"], np.float32)
    w_v = np.asarray(inputs["w_v"], np.float32)
    w_o = np.asarray(inputs["w_o"], np.float32)
    rf = np.asarray(inputs["rope_freqs"], np.float32)  # [T, 32, 2]

    # q head permutation: chunk c = (PI[2c], PI[2c+1])
    perm_rows = np.empty(1024, np.int64)
    for c in range(8):
        for j in range(128):
            h = PI[2 * c + (1 if j >= 64 else 0)]
            perm_rows[128 * c + j] = 64 * h + (j % 64)
    wq_perm = w_q[perm_rows]

    wqT = np.ascontiguousarray(wq_perm.T).astype(bf)          # [1024d, 1024o]
    wkT = np.ascontiguousarray(w_k.T).astype(bf)              # [1024, 256]
    wvT = np.ascontiguousarray(w_v.T).astype(bf)              # [1024, 256]
    woT = np.ascontiguousarray(w_o.T).astype(bf)              # [1024o, 1024n]

    # rope tables [128, T]: partition p -> dim (p % 64) of any head
    p = np.arange(128)
    f = (p % 64) // 2
    sign = np.where(p % 2 == 1, 1.0, -1.0).astype(np.float32)
    c2k = np.ascontiguousarray(rf[:, f, 0].T).astype(bf)          # [128, T]
    s2k = np.ascontiguousarray((rf[:, f, 1] * sign).T).astype(bf)

    swap_m = np.zeros((128, 128), np.float32)
    q = (p // 64) * 64 + ((p % 64) ^ 1)
    swap_m[q, p] = 1.0
    swap_m = swap_m.astype(bf)

    in_maps = []
    for c in range(8):
        b, s = divmod(c, 4)
        J = [s, 7 - s, 8 + s, 15 - s]
        tcols = np.concatenate([np.arange(128 * j, 128 * (j + 1)) for j in J])
        xTb = np.ascontiguousarray(x[b].T).astype(bf)         # [1024, 2048]
        xqT = np.ascontiguousarray(xTb[:, tcols])             # [1024, 512]
        c2q = np.ascontiguousarray(c2k[:, tcols])
        s2q = np.ascontiguousarray(s2k[:, tcols])
        bias_t = np.zeros((4, 16), np.float32)
        for v in range(4):
            bias_t[v, J[v]:] = NEG   # off-diag loop: block i valid iff i < J[v]
        bias_full = np.ascontiguousarray(
            np.broadcast_to(bias_t.reshape(1, 64), (128, 64)))
        in_maps.append({
            "xT": xTb, "xqT": xqT, "wqT": wqT, "wkT": wkT, "wvT": wvT,
            "woT": woT, "c2k": c2k, "s2k": s2k, "c2q": c2q, "s2q": s2q,
            "swap": swap_m, "bias": bias_full,
        })
    return in_maps


def run(trace=False, tmpdir=None, **inputs):
    nc = _build()
    in_maps = _host_prep(inputs)
    res = run_bass_kernel_spmd(nc, in_maps, core_ids=list(range(8)), trace=trace,
                               tmpdir=tmpdir)
    out = np.empty((B, T, D), np.float32)
    for c in range(8):
        b, s = divmod(c, 4)
        J = [s, 7 - s, 8 + s, 15 - s]
        for v in range(4):
            out[b, 128 * J[v]:128 * (J[v] + 1)] = \
                res.results[c]["out"][128 * v:128 * (v + 1)]
    return out, res


def kernel(**inputs):
    out, _ = run(trace=False, **inputs)
    return out


# revision 16
# speedup vs baseline: 1.4141x; 1.1081x over previous
"""GQA attention kernel for 8 Trainium2 NeuronCores (Bass/Tile).

Sharding: data-parallel over batch (2) x sequence-parallel over query blocks.
Core c = (b, s): batch b = c//4, slice s = c%4 owns query blocks
J = [s, 7-s, 8+s, 15-s] (128 rows each; causal work is exactly balanced:
sum of block costs = 34 for every s). Each core computes K/V for the full
sequence (cheap duplication) -> zero collectives; output is a pure
concatenation on the host.

The program is core-uniform (SPMD). Per slot v the off-diagonal key-block
loop is padded to PBAR[v]-1 = [3,7,11,15] blocks; out-of-range blocks are
killed by a per-core exp bias table (exp(s/8 - 50) ~ 0). The causal
diagonal block is a separate fixed iteration whose K/V come from an extra
projection of the per-core gathered query columns (xqT), so its mask is a
core-independent affine_select.

All weights (transposed, bf16), x (transposed, bf16), gathered q-columns,
RoPE cos/sin tables (expanded, signed) and the pair-swap matrix are
prepared host-side in run().

Hardcoded problem: B=2 T=2048 D=1024 n_heads=16 n_kv=4 d_head=64, causal,
rope passed as input, scale=1/8.
"""

import numpy as np
import ml_dtypes

import concourse.bass as bass
import concourse.tile as tile
from concourse import bacc, mybir
from concourse.bass_utils import run_bass_kernel_spmd

F32 = mybir.dt.float32
BF16 = mybir.dt.bfloat16

B, T, D = 2, 2048, 1024
NH, NKV, DH = 16, 4, 64
NTB = T // 128            # 16 key blocks
PBAR = [4, 8, 12, 16]     # total key-block count per slot (uniform program)
SCALE = 1.0 / 8.0
NEG = -50.0               # exp bias for out-of-range blocks

# head permutation: q-chunk c holds heads (PI[2c], PI[2c+1]) in its two
# 64-partition halves -> each chunk pairs a g-even head with a g-odd head so
# scores can be row-tiled (two concurrent K=64 matmuls) against the matching
# kv-head pair.
PI = [0, 4, 1, 5, 2, 6, 3, 7, 8, 12, 9, 13, 10, 14, 11, 15]

USE_APPROX_RECIP = True

_CACHE = {}


def _emit(nc, tc, aps):
    (xT_ap, xqT_ap, wqT_ap, wkT_ap, wvT_ap, woT_ap,
     c2k_ap, s2k_ap, c2q_ap, s2q_ap, swap_ap, bias_ap, tri_ap, out_ap) = aps
    import contextlib
    ctx = contextlib.ExitStack()
    with ctx:
        sing = ctx.enter_context(tc.tile_pool(name="sing", bufs=1))
        qsbp = ctx.enter_context(tc.tile_pool(name="qsbp", bufs=4))
        ropet = ctx.enter_context(tc.tile_pool(name="ropet", bufs=4))
        ptp = ctx.enter_context(tc.tile_pool(name="ptp", bufs=4))
        rp = ctx.enter_context(tc.tile_pool(name="rp", bufs=2))
        osp = ctx.enter_context(tc.tile_pool(name="osp", bufs=3))
        # PSUM: tp(3x2 banks) + oap(2x1) = 8 banks
        tp = ctx.enter_context(tc.tile_pool(name="tp", bufs=3, space="PSUM"))
        oap = ctx.enter_context(tc.tile_pool(name="oap", bufs=2, space="PSUM"))

        # ---- persistent SBUF tensors; DMAs emitted in dependency priority
        xT = sing.tile([128, 8, T], BF16)
        wkT = sing.tile([128, 8, 256], BF16)
        wvT = sing.tile([128, 8, 256], BF16)
        wqT = sing.tile([128, 8, 1024], BF16)
        woT = sing.tile([64, 16, 1024], BF16)
        xqT = sing.tile([128, 8, 512], BF16)
        c2k = sing.tile([128, T], BF16)
        s2k = sing.tile([128, T], BF16)
        c2q = sing.tile([128, 512], BF16)
        s2q = sing.tile([128, 512], BF16)
        swap = sing.tile([128, 128], BF16)
        bias = sing.tile([128, 4, 16], F32)

        nc.sync.dma_start(wkT[:], wkT_ap.rearrange("(m p) o -> p m o", p=128))
        nc.sync.dma_start(swap[:], swap_ap[:, :])
        tri = sing.tile([128, 128], BF16)
        nc.sync.dma_start(tri[:], tri_ap[:, :])
        nc.sync.dma_start(c2k[:, 0:512], c2k_ap[:, 0:512])
        nc.sync.dma_start(s2k[:, 0:512], s2k_ap[:, 0:512])
        for m in range(8):  # ts0 columns of x first
            nc.sync.dma_start(xT[:, m, 0:512], xT_ap[128 * m:128 * (m + 1), 0:512])
        nc.sync.dma_start(xqT[:], xqT_ap.rearrange("(m p) t -> p m t", p=128))
        nc.sync.dma_start(c2q[:], c2q_ap[:, :])
        nc.sync.dma_start(s2q[:], s2q_ap[:, :])
        nc.sync.dma_start(wvT[:], wvT_ap.rearrange("(m p) o -> p m o", p=128))
        nc.sync.dma_start(wqT[:], wqT_ap.rearrange("(m p) o -> p m o", p=128))
        nc.sync.dma_start(bias[:], bias_ap.rearrange("p (v i) -> p v i", v=4))
        for ts in range(1, 4):
            for m in range(8):
                nc.sync.dma_start(xT[:, m, 512 * ts:512 * (ts + 1)],
                                  xT_ap[128 * m:128 * (m + 1),
                                        512 * ts:512 * (ts + 1)])
            nc.sync.dma_start(c2k[:, 512 * ts:512 * (ts + 1)],
                              c2k_ap[:, 512 * ts:512 * (ts + 1)])
            nc.sync.dma_start(s2k[:, 512 * ts:512 * (ts + 1)],
                              s2k_ap[:, 512 * ts:512 * (ts + 1)])
        nc.sync.dma_start(woT[:], woT_ap.rearrange("(h p) n -> p h n", p=64))

        kroped = sing.tile([128, 2, T], BF16)     # [kv-pair half, P, t]
        kdiag = sing.tile([128, 2, 4, 128], BF16)  # [half, P, slot, tq]
        qTd = sing.tile([128, 8, 4, 128], BF16)   # [head-pair half, chunk, slot, tq]
        vaug = sing.tile([128, NTB, 4, 65], BF16)  # [tk, tb, g, dh | ones]
        nc.vector.memset(vaug[:], 1.0)
        vdiag = sing.tile([128, 4, 4, 65], BF16)  # [tk, slot, g, dh | ones]
        nc.vector.memset(vdiag[:], 1.0)
        oaTn = sing.tile([64, 16, 4, 128], BF16)  # [dh, head, slot, tq]

        def rope(kt, dst, cos_ap, sin_ap):
            """kt: [128, 1024] psum tile; raw proj in [:, 0:512], the
            swap-matmul result goes into [:, 512:1024].
            dst/cos/sin: [128, 512] bf16 views."""
            qsb = qsbp.tile([128, 512], BF16, tag="qsb")
            nc.scalar.copy(qsb[:], kt[:, 0:512])
            nc.tensor.matmul(kt[:, 512:1024], swap[:], qsb[:],
                             start=True, stop=True)
            t1 = ropet.tile([128, 512], BF16, tag="t1")
            nc.vector.tensor_mul(t1[:], qsb[:], cos_ap)
            t2 = ropet.tile([128, 512], F32, tag="t2")
            nc.vector.tensor_mul(t2[:], kt[:, 512:1024], sin_ap)
            nc.vector.tensor_add(dst, t1[:], t2[:])

        def kproj(och, ts):
            kt = tp.tile([128, 1024], F32, tag="t")
            for m in range(8):
                nc.tensor.matmul(kt[:, 0:512], wkT[:, m, 128 * och:128 * (och + 1)],
                                 xT[:, m, 512 * ts:512 * (ts + 1)],
                                 start=(m == 0), stop=(m == 7))
            rope(kt, kroped[:, och, 512 * ts:512 * (ts + 1)],
                 c2k[:, 512 * ts:512 * (ts + 1)],
                 s2k[:, 512 * ts:512 * (ts + 1)])

        def vproj(tb):
            vt = tp.tile([128, 1024], F32, tag="t")
            for m in range(8):
                nc.tensor.matmul(vt[:, 0:256], xT[:, m, 128 * tb:128 * (tb + 1)],
                                 wvT[:, m, :], start=(m == 0), stop=(m == 7))
            nc.scalar.copy(vaug[:, tb, :, 0:64],
                           vt[:, 0:256].rearrange("p (g d) -> p g d", g=4))

        def kdiagproj(och):
            kt = tp.tile([128, 1024], F32, tag="t")
            for m in range(8):
                nc.tensor.matmul(kt[:, 0:512], wkT[:, m, 128 * och:128 * (och + 1)],
                                 xqT[:, m, :], start=(m == 0), stop=(m == 7))
            rope(kt, kdiag[:, och, :, :].rearrange("p v t -> p (v t)"),
                 c2q[:, :], s2q[:, :])

        def vdiagproj(v):
            vt = tp.tile([128, 1024], F32, tag="t")
            for m in range(8):
                nc.tensor.matmul(vt[:, 0:256], xqT[:, m, 128 * v:128 * (v + 1)],
                                 wvT[:, m, :], start=(m == 0), stop=(m == 7))
            nc.scalar.copy(vdiag[:, v, :, 0:64],
                           vt[:, 0:256].rearrange("p (g d) -> p g d", g=4))

        def qproj(och):
            qt = tp.tile([128, 1024], F32, tag="t")
            for m in range(8):
                nc.tensor.matmul(qt[:, 0:512], wqT[:, m, 128 * och:128 * (och + 1)],
                                 xqT[:, m, :], start=(m == 0), stop=(m == 7))
            rope(qt, qTd[:, och, :, :].rearrange("p v t -> p (v t)"),
                 c2q[:, :], s2q[:, :])

        def attend(v, P):
            oaA = oap.tile([128, 512], F32, tag="oa", name=f"oaA{v}{P}")
            oaB = oap.tile([128, 512], F32, tag="oa", name=f"oaB{v}{P}")
            rhsA = qTd[0:64, 4 * P:4 * P + 4, v, :]
            rhsB = qTd[64:128, 4 * P:4 * P + 4, v, :]
            # diagonal block (own columns, fixed causal mask)
            st = tp.tile([128, 1024], F32, tag="t")
            nc.tensor.matmul(st[:, 0:512], kdiag[0:64, P, v, :], rhsA,
                             start=True, stop=True)
            nc.tensor.matmul(st[:, 512:1024], kdiag[64:128, P, v, :], rhsB,
                             start=True, stop=True, tile_position=(64, 0))
            pt = ptp.tile([128, 1024], BF16, tag="pt")
            nc.scalar.activation(pt[:], st[:], mybir.ActivationFunctionType.Exp,
                                 scale=SCALE)
            tv = tri[:]
            trib = bass.AP(tensor=tv.tensor, offset=tv.offset,
                           ap=[tv.ap[0], [0, 8], tv.ap[1]])
            nc.vector.tensor_mul(pt[:].rearrange("p (h c) -> p h c", h=8),
                                 pt[:].rearrange("p (h c) -> p h c", h=8),
                                 trib)
            nc.tensor.matmul(oaA[0:65, :], vdiag[:, v, 2 * P, :], pt[:, 0:512],
                             start=True, stop=False)
            nc.tensor.matmul(oaB[0:65, :], vdiag[:, v, 2 * P + 1, :],
                             pt[:, 512:1024], start=True, stop=False)
            # off-diagonal blocks (bias table kills i >= J[v])
            nb = PBAR[v] - 1
            for i in range(nb):
                st = tp.tile([128, 1024], F32, tag="t")
                nc.tensor.matmul(st[:, 0:512],
                                 kroped[0:64, P, 128 * i:128 * (i + 1)], rhsA,
                                 start=True, stop=True)
                nc.tensor.matmul(st[:, 512:1024],
                                 kroped[64:128, P, 128 * i:128 * (i + 1)], rhsB,
                                 start=True, stop=True, tile_position=(64, 0))
                pt = ptp.tile([128, 1024], BF16, tag="pt")
                nc.scalar.activation(pt[:], st[:],
                                     mybir.ActivationFunctionType.Exp,
                                     scale=SCALE, bias=bias[:, v, i:i + 1])
                nc.tensor.matmul(oaA[0:65, :], vaug[:, i, 2 * P, :],
                                 pt[:, 0:512], start=False, stop=(i == nb - 1))
                nc.tensor.matmul(oaB[0:65, :], vaug[:, i, 2 * P + 1, :],
                                 pt[:, 512:1024], start=False, stop=(i == nb - 1))
            for half, oa in ((0, oaA), (1, oaB)):
                rsb = rp.tile([1, 512], F32, tag="rsb")
                if USE_APPROX_RECIP:
                    dcp = rp.tile([1, 512], F32, tag="dcp")
                    nc.scalar.copy(dcp[:], oa[64:65, :])
                    nc.vector.reciprocal_approx_fast(out=rsb[:], in_=dcp[:])
                else:
                    nc.vector.reciprocal(rsb[:], oa[64:65, :])
                rb = rp.tile([64, 512], F32, tag="rb")
                nc.gpsimd.partition_broadcast(rb[:], rsb[:])
                h0 = 8 * P + 4 * half
                nc.vector.tensor_mul(
                    oaTn[0:64, h0:h0 + 4, v, :],
                    oa[0:64, :].rearrange("p (h c) -> p h c", h=4),
                    rb[:].rearrange("p (h c) -> p h c", h=4))

        def outproj(v):
            op = tp.tile([128, 1024], F32, tag="t")
            for nh in range(2):
                for h in range(16):
                    nc.tensor.matmul(op[:, 512 * nh:512 * (nh + 1)],
                                     oaTn[0:64, h, v, :],
                                     woT[0:64, h, 512 * nh:512 * (nh + 1)],
                                     start=(h == 0), stop=(h == 15))
            for nh in range(2):
                ob = osp.tile([128, 512], F32, tag="ob")
                nc.vector.tensor_copy(ob[:], op[:, 512 * nh:512 * (nh + 1)])
                nc.sync.dma_start(
                    out_ap[128 * v:128 * (v + 1), 512 * nh:512 * (nh + 1)], ob[:])

        # minimal preamble for attend(0,0): kv-pair 0 K, diag K/V, q-chunks 0-3
        kproj(0, 0)
        kdiagproj(0)
        for och in range(4):
            qproj(och)
        vdiagproj(0)
        for tb in range(3):
            vproj(tb)
        attend(0, 0)
        kproj(1, 0)
        kdiagproj(1)
        for och in range(4, 8):
            qproj(och)
        for v in range(1, 4):
            vdiagproj(v)
        attend(0, 1)
        kproj(0, 1)
        kproj(1, 1)
        for tb in range(3, 7):
            vproj(tb)
        outproj(0)
        for v in range(1, 4):
            attend(v, 0)
            attend(v, 1)
            if v < 3:  # next slice's K/V
                kproj(0, v + 1)
                kproj(1, v + 1)
                for tb in range(4 * v + 3, 4 * v + 7):
                    vproj(tb)
            outproj(v)


def _build():
    if "nc" in _CACHE:
        return _CACHE["nc"]
    nc = bacc.Bacc("TRN2", target_bir_lowering=False, debug=False, num_devices=8)
    aps = (
        nc.dram_tensor("xT", [D, T], BF16, kind="ExternalInput").ap(),
        nc.dram_tensor("xqT", [D, 512], BF16, kind="ExternalInput").ap(),
        nc.dram_tensor("wqT", [D, 1024], BF16, kind="ExternalInput").ap(),
        nc.dram_tensor("wkT", [D, 256], BF16, kind="ExternalInput").ap(),
        nc.dram_tensor("wvT", [D, 256], BF16, kind="ExternalInput").ap(),
        nc.dram_tensor("woT", [1024, 1024], BF16, kind="ExternalInput").ap(),
        nc.dram_tensor("c2k", [128, T], BF16, kind="ExternalInput").ap(),
        nc.dram_tensor("s2k", [128, T], BF16, kind="ExternalInput").ap(),
        nc.dram_tensor("c2q", [128, 512], BF16, kind="ExternalInput").ap(),
        nc.dram_tensor("s2q", [128, 512], BF16, kind="ExternalInput").ap(),
        nc.dram_tensor("swap", [128, 128], BF16, kind="ExternalInput").ap(),
        nc.dram_tensor("bias", [128, 64], F32, kind="ExternalInput").ap(),
        nc.dram_tensor("tri", [128, 128], BF16, kind="ExternalInput").ap(),
        nc.dram_tensor("out", [512, D], F32, kind="ExternalOutput").ap(),
    )
    with tile.TileContext(nc) as tc:
        _emit(nc, tc, aps)
    nc.compile()
    _CACHE["nc"] = nc
    return nc


def _host_prep(inputs):
    """Build per-core input maps (all numpy, untimed host prep)."""
    bf = ml_dtypes.bfloat16
    x = np.asarray(inputs["x"], np.float32)
    w_q = np.asarray(inputs["w_q"], np.float32)
    w_k = np.asarray(inputs["w_k"], np.float32)
    w_v = np.asarray(inputs["w_v"], np.float32)
    w_o = np.asarray(inputs["w_o"], np.float32)
    rf = np.asarray(inputs["rope_freqs"], np.float32)  # [T, 32, 2]

    # q head permutation: chunk c = (PI[2c], PI[2c+1])
    perm_rows = np.empty(1024, np.int64)
    for c in range(8):
        for j in range(128):
            h = PI[2 * c + (1 if j >= 64 else 0)]
            perm_rows[128 * c + j] = 64 * h + (j % 64)
    wq_perm = w_q[perm_rows]

    wqT = np.ascontiguousarray(wq_perm.T).astype(bf)          # [1024d, 1024o]
    wkT = np.ascontiguousarray(w_k.T).astype(bf)              # [1024, 256]
    wvT = np.ascontiguousarray(w_v.T).astype(bf)              # [1024, 256]
    woT = np.ascontiguousarray(w_o.T).astype(bf)              # [1024o, 1024n]

    # rope tables [128, T]: partition p -> dim (p % 64) of any head
    p = np.arange(128)
    f = (p % 64) // 2
    sign = np.where(p % 2 == 1, 1.0, -1.0).astype(np.float32)
    c2k = np.ascontiguousarray(rf[:, f, 0].T).astype(bf)          # [128, T]
    s2k = np.ascontiguousarray((rf[:, f, 1] * sign).T).astype(bf)

    tri_m = (np.arange(128)[None, :] >= np.arange(128)[:, None]).astype(bf)
    swap_m = np.zeros((128, 128), np.float32)
    q = (p // 64) * 64 + ((p % 64) ^ 1)
    swap_m[q, p] = 1.0
    swap_m = swap_m.astype(bf)

    in_maps = []
    for c in range(8):
        b, s = divmod(c, 4)
        J = [s, 7 - s, 8 + s, 15 - s]
        tcols = np.concatenate([np.arange(128 * j, 128 * (j + 1)) for j in J])
        xTb = np.ascontiguousarray(x[b].T).astype(bf)         # [1024, 2048]
        xqT = np.ascontiguousarray(xTb[:, tcols])             # [1024, 512]
        c2q = np.ascontiguousarray(c2k[:, tcols])
        s2q = np.ascontiguousarray(s2k[:, tcols])
        bias_t = np.zeros((4, 16), np.float32)
        for v in range(4):
            bias_t[v, J[v]:] = NEG   # off-diag loop: block i valid iff i < J[v]
        bias_full = np.ascontiguousarray(
            np.broadcast_to(bias_t.reshape(1, 64), (128, 64)))
        in_maps.append({
            "xT": xTb, "xqT": xqT, "wqT": wqT, "wkT": wkT, "wvT": wvT,
            "woT": woT, "c2k": c2k, "s2k": s2k, "c2q": c2q, "s2q": s2q,
            "swap": swap_m, "bias": bias_full, "tri": tri_m,
        })
    return in_maps


def run(trace=False, tmpdir=None, **inputs):
    nc = _build()
    in_maps = _host_prep(inputs)
    res = run_bass_kernel_spmd(nc, in_maps, core_ids=list(range(8)), trace=trace,
                               tmpdir=tmpdir)
    out = np.empty((B, T, D), np.float32)
    for c in range(8):
        b, s = divmod(c, 4)
        J = [s, 7 - s, 8 + s, 15 - s]
        for v in range(4):
            out[b, 128 * J[v]:128 * (J[v] + 1)] = \
                res.results[c]["out"][128 * v:128 * (v + 1)]
    return out, res


def kernel(**inputs):
    out, _ = run(trace=False, **inputs)
    return out
